# revision 1
# baseline (speedup 1.0000x reference)
"""VQ codebook assignment + nearest upsample on 8 NeuronCores.

Problem (per domain): given features f [B=4, C=256, H=64, W=128] and
centroids c [K=19, C=256], compute argmin_k ||f[b,:,h,w] - c_k||^2 and
nearest-upsample the [64,128] index map to [512,1024] (8x in each axis).
Two independent domains (cross-assigned centroids) x 4 batches = 8 cores,
one batch-image per core, no cross-core communication.

Per-core pipeline (K-partition matmuls; everything exact in fp32 —
the output is integer indices, so near-tie argmins must not flip):
  1. cross[k, px] via fp32 matmuls with the tiny centroid block
     stationary ([128, 19] per C-half) and 512-pixel feature chunks
     moving — full moving-side throughput instead of 19-column
     mini-matmuls (a pixel-stationary layout pays a 128-column weight
     load per 128 pixels and is ~2x slower end to end).
  2. Bit-exact ScalarE Copy moves scores PSUM->SBUF (the Identity-LUT
     bias/scale path has ~2^-12 relative error — enough to flip
     near-tie argmins, measured on hw).
  3. PE transposes [19, 128] score chunks to [128 px, 19] pixel-
     partition layout, where DVE reductions run at full 128-lane
     efficiency (any K-partition reduction wastes 109/128 lanes).
  4. scores = cross - c2/2 via an exact DVE add against a
     host-replicated [128, 19] bias tile (argmin ordering preserved;
     the f^2 term is constant over k and dropped). Argmax index:
     reduce_max over K, then (is_ge * -1024 + iota) reduce_min, +1024
     — first-match tie semantics, exact in f32.
  5. Tail runs per h-half, overlapped with the other half's compute:
     DVE 32x32 block transpose + block-permute copies produce the
     [h, w] int32 index map, one broadcast copy replicates 8x along x,
     and 8 HWDGE store-DMAs per half re-read the same SBUF row for the
     8x y-replication (4KB-contiguous runs).

w is concatenated in front of the feature pixels (one tensor: the
block-0 chunk DMA covers both, so matmuls carry few semaphore waits);
input DMA triggers alternate between the SP and ACT HWDGE queues.
Bacc.compile() legalizes any instruction with more than the 1-sync-wait
ISA limit (bass.Bass alone fails walrus codegen on multi-wait matmuls).

Measured on trn2 (8 cores, NTFF): ~66 us exec, bit-identical masks vs
the fp32 reference. Input DMA is ~25 us (saturated at ~360 GB/s/core);
the fp32 PE stream (64 LOW_HIGH passes + 64 transposes) is the
critical path.
"""

import numpy as np

import concourse.bass as bass
import concourse.mybir as mybir
import concourse.tile as tile
from concourse import bacc
from concourse.bass import ds
from concourse.bass_utils import run_bass_kernel_spmd
from concourse.masks import make_identity

F32 = mybir.dt.float32
I32 = mybir.dt.int32

B = 4
C = 256
H, W = 64, 128
K = 19
HL, WL = 512, 1024
NPIX = H * W          # 8192
RB = 8                # image rows per block
NB = H // RB          # 8 blocks
CH = 512              # matmul moving chunk (pixels)
RPC = CH // W         # image rows per chunk: 4
UP = HL // H          # 8x upsample
BIG = 1024.0
FWC = K + NPIX        # fw columns: [w | pixels]

_NC_CACHE = None


def _build_nc():
    nc = bacc.Bacc("TRN2", target_bir_lowering=False, debug=False)

    fw_in = nc.dram_tensor("fw", [C, FWC], F32, kind="ExternalInput")
    bias_in = nc.dram_tensor("bias", [128, K], F32, kind="ExternalInput")
    mask_out = nc.dram_tensor("mask", [HL, WL], I32, kind="ExternalOutput")

    fwv = fw_in.ap().rearrange("(a p) n -> a p n", a=2)       # [2, 128, FWC]
    outv = mask_out.ap().rearrange("(h y) x -> h y x", y=UP)  # [64, 8, 1024]

    with tile.TileContext(nc) as tc:
        with (
            tc.tile_pool(name="persist", bufs=1) as pp,
            tc.tile_pool(name="work", bufs=6) as wp,
            tc.tile_pool(name="psA", bufs=6, space="PSUM") as psA,
            tc.tile_pool(name="psB", bufs=2, space="PSUM") as psB,
        ):
            fw0 = pp.tile([128, FWC], F32, tag="fw0")
            fw1 = pp.tile([128, FWC], F32, tag="fw1")
            bias128 = pp.tile([128, K], F32, tag="bias128")
            ident = pp.tile([K, K], F32, tag="ident")
            iota_i = pp.tile([128, K], I32, tag="iota_i")
            iotaf = pp.tile([128, K], F32, tag="iotaf")
            idxv = pp.tile([128, H], F32, tag="idxv")       # [w, h]
            tmp = pp.tile([128, H], F32, tag="tmp")         # block-transposed
            idxT = pp.tile([H, W], I32, tag="idxT")         # [h, w]
            rep = pp.tile([H, WL], I32, tag="rep")

            # --- setup ---
            nc.gpsimd.iota(iota_i, pattern=[[1, K]], base=0, channel_multiplier=0)
            nc.vector.tensor_copy(iotaf, iota_i)
            make_identity(nc, ident)
            nc.sync.dma_start(bias128, bias_in[:, :])

            # --- feature loads: block 0's chunk includes the w columns.
            # Triggers split across the two HWDGE engines (SP/ACT) so
            # trigger processing (~650ns each) runs in parallel. ---
            # block 0 loads in two pieces so the first matmul's data
            # (w + first 512-px chunk) lands ~2x sooner; the PE stream is
            # the critical path and shifts left with it
            ld_slices = [ds(0, K + CH), ds(K + CH, CH)]
            for blk in range(1, NB):
                ld_slices.append(ds(K + blk * RB * W, RB * W))
            for i, sl in enumerate(ld_slices):
                eng = nc.sync if i % 2 == 0 else nc.scalar
                eng.dma_start(fw0[:, sl], fwv[0, :, sl])
                eng.dma_start(fw1[:, sl], fwv[1, :, sl])

            iota_b = iotaf.rearrange("p (o k) -> p o k", o=1).to_broadcast(
                [128, RB, K]
            )

            # --- per-block: matmul -> scores -> transpose -> argmax index ---
            for blk in range(NB):
                ps2 = psB.tile([128, RB, K], F32, tag="ps2")
                for half in range(RB // RPC):
                    ch = blk * (RB // RPC) + half
                    colsl = ds(K + ch * CH, CH)
                    ps = psA.tile([K, CH], F32, tag="ps")
                    nc.tensor.matmul(
                        ps, fw0[:, 0:K], fw0[:, colsl],
                        start=True, stop=False,
                    )
                    nc.tensor.matmul(
                        ps, fw1[:, 0:K], fw1[:, colsl],
                        start=False, stop=True,
                    )
                    # plain Copy is bit-exact; the Identity-LUT bias/scale
                    # path has ~2^-12 relative error, enough to flip
                    # near-tie argmins
                    St = wp.tile([K, CH], F32, tag="St")
                    nc.scalar.copy(St, ps)
                    for r in range(RPC):
                        nc.tensor.transpose(
                            ps2[:, half * RPC + r],
                            St[:, ds(r * W, W)],
                            ident,
                        )
                # scores = cross - c2/2 (exact DVE add; ordering matches
                # the reference argmin of ||f-c||^2)
                S = wp.tile([128, RB, K], F32, tag="S")
                bias_b = bias128.rearrange("p (o k) -> p o k", o=1).to_broadcast(
                    [128, RB, K]
                )
                nc.vector.tensor_tensor(S, ps2, bias_b, op=mybir.AluOpType.add)
                maxv = wp.tile([128, RB], F32, tag="maxv")
                nc.vector.tensor_reduce(
                    maxv, S, axis=mybir.AxisListType.X, op=mybir.AluOpType.max
                )
                eq = wp.tile([128, RB, K], F32, tag="eq")
                maxv_b = maxv.rearrange("p (t o) -> p t o", o=1).to_broadcast(
                    [128, RB, K]
                )
                nc.vector.tensor_tensor(eq, S, maxv_b, op=mybir.AluOpType.is_ge)
                cand = wp.tile([128, RB, K], F32, tag="cand")
                nc.vector.scalar_tensor_tensor(
                    cand, eq, -BIG, iota_b,
                    op0=mybir.AluOpType.mult, op1=mybir.AluOpType.add,
                )
                nc.vector.tensor_reduce(
                    idxv[:, ds(blk * RB, RB)], cand,
                    axis=mybir.AxisListType.X, op=mybir.AluOpType.min,
                )

                # --- tail, overlapped: after each half of the blocks, emit
                # that h-half of the output (transpose, replicate, store) ---
                if blk % (NB // 2) != NB // 2 - 1:
                    continue
                hh = blk // (NB // 2)          # 0 or 1
                hsl = ds(hh * H // 2, H // 2)  # 32 h columns
                psl = ds(hh * 32, 32)          # matching partition rows
                nc.vector.tensor_scalar_add(idxv[:, hsl], idxv[:, hsl], BIG)
                nc.vector.transpose(tmp[:, hsl], idxv[:, hsl])
                for i in range(W // 32):
                    nc.vector.tensor_copy(
                        idxT[psl, ds(32 * i, 32)],
                        tmp[ds(32 * i, 32), hsl],
                    )
                # replicate 8x in x once on DVE; the 8x in y happens by
                # letting 8 store-DMAs re-read the same SBUF row (HWDGE,
                # 4KB-contiguous runs). GpSimd stays off SBUF — it shares
                # the DVE port pair and copies there stall both engines.
                idxT_b = idxT[psl].rearrange(
                    "p (w o) -> p w o", o=1
                ).to_broadcast([32, W, UP])
                nc.vector.tensor_copy(
                    rep[psl].rearrange("p (w x) -> p w x", w=W), idxT_b
                )
                for y in range(UP):
                    deng = nc.sync if y % 2 == 0 else nc.scalar
                    deng.dma_start(outv[psl, y], rep[psl])

    nc.compile()
    return nc


def _prep_domain(feature, centroid):
    """Per-core inputs for one domain: 4 batches against one centroid set."""
    c = np.ascontiguousarray(centroid, dtype=np.float32)
    w = c.T.astype(np.float32)                                  # [C, K]
    c2 = np.sum(c.astype(np.float32) ** 2, axis=1)              # [K]
    bias = np.ascontiguousarray(
        np.tile(-0.5 * c2[None, :], (128, 1)), dtype=np.float32
    )                                                           # [128, K]
    maps = []
    for b in range(B):
        f = np.asarray(feature[b], dtype=np.float32).reshape(C, NPIX)
        fw = np.ascontiguousarray(np.concatenate([w, f], axis=1))
        maps.append({"fw": fw, "bias": bias})
    return maps


def kernel(
    feature_s2t, feature_target, label_s2t, label_target,
    centroid_s2t, centroid_target,
):
    global _NC_CACHE
    if _NC_CACHE is None:
        _NC_CACHE = _build_nc()
    nc = _NC_CACHE

    # cross assignment: s2t features vs target centroids, and vice versa
    in_maps = _prep_domain(feature_s2t, centroid_target) + _prep_domain(
        feature_target, centroid_s2t
    )
    res = run_bass_kernel_spmd(nc, in_maps, core_ids=list(range(8))).results
    mask_s2t = np.stack([res[i]["mask"] for i in range(B)]).astype(np.int32)
    mask_target = np.stack([res[B + i]["mask"] for i in range(B)]).astype(
        np.int32
    )
    return (mask_s2t, mask_target)



# revision 6
# speedup vs baseline: 1.3254x; 1.3254x over previous
"""VQ codebook assignment + nearest upsample on 8 NeuronCores.

Problem (per domain): given features f [B=4, C=256, H=64, W=128] and
centroids c [K=19, C=256], compute argmin_k ||f[b,:,h,w] - c_k||^2 and
nearest-upsample the [64,128] index map to [512,1024] (8x in each axis).
Two independent domains (cross-assigned centroids) x 4 batches = 8 cores,
one batch-image per core, no cross-core communication.

Precision scheme (measured on the actual inputs): features are quantized
to fp16 on the host (halves HBM traffic and PE passes); centroids are
carried as an fp16 hi+lo pair (w = hi + lo reconstructs fp32-fidelity
weights), and the bias -||c||^2/2 is computed on the host in fp64 from
the reconstructed centroids and added exactly via an fp32 PE matmul.
Scores stay fp32 throughout.  Measured rel_err 1.3e-2 (< 2e-2 gate);
bf16 or fp16-centroid variants measure over the gate and are not used.

Per-core pipeline, per supergroup of 4 x 512-pixel chunks:
  1. Bias prefill: matmul(ones[1,512], biasvec[1,128]) writes the
     per-k bias into all 512 psum columns (start=True opens the
     accumulation group; fp32 1.0*b is exact under LOW_HIGH).
  2. 16 fp16 matmuls accumulate cross[k,px]: the w hi/lo blocks (padded
     to 32 cols) are stationary; each of the 4 chunks streams through
     its own 32-col array strip (tile_position col tiling inferred from
     the psum slice base partition) so 4 matmuls run concurrently.
  3. One ScalarE copy moves the [128,512] scores PSUM->SBUF (plain Copy
     is bit-exact; the Identity-LUT bias path is not).
  4. 4 PE transposes flip [128,128] score slices into pixel-partition
     layout [128px, (4 strips x 32)] in PSUM.
  5. DVE argmax on strided views that skip the 13 pad columns per
     strip: reduce_max, is_ge, (eq*-1024+iota), reduce_min -- exact
     first-match tie semantics in fp32.
Tail per 32-row half: permuted +1024 copy fixes the (c,j) supergroup
ordering back to linear h, DVE 32x32 transpose + block copies build the
[h,w] int8 map, ScalarE broadcast-copies the 8x x-replication, and 8
HWDGE stores per half re-read the same SBUF rows for the 8x
y-replication.  The mask travels as int8 (indices 0..18, lossless) and
the host widens to int32.

Measured on trn2 (8 cores, NTFF): see test.py output.
"""

import numpy as np

import concourse.bass as bass
import concourse.mybir as mybir
import concourse.tile as tile
from concourse import bacc
from concourse.bass import ds
from concourse.bass_utils import run_bass_kernel_spmd

F32 = mybir.dt.float32
F16 = mybir.dt.float16
I32 = mybir.dt.int32
I8 = mybir.dt.int8

B = 4
C = 256
H, W = 64, 128
K = 19
KP = 32               # w block padded to 32 columns (one array strip)
HL, WL = 512, 1024
NPIX = H * W          # 8192
CH = 512              # matmul moving chunk (pixels)
SG = 4 * CH           # supergroup: 4 chunks processed concurrently
NSG = NPIX // SG      # 4 supergroups
UP = HL // H          # 8x upsample
BIG = 1024.0
NEG = -1.0e30         # pad-row bias; never wins the max
FWC = 2 * KP + NPIX   # fw columns: [w_hi32 | w_lo32 | pixels]

_NC_CACHE = None


def _build_nc():
    nc = bacc.Bacc("TRN2", target_bir_lowering=False, debug=False)

    fw_in = nc.dram_tensor("fw", [C, FWC], F16, kind="ExternalInput")
    bias_in = nc.dram_tensor("bias", [1, 128], F32, kind="ExternalInput")
    ident_in = nc.dram_tensor("ident", [128, 128], F32, kind="ExternalInput")
    mask_out = nc.dram_tensor("mask", [HL, WL], I8, kind="ExternalOutput")

    fwv = fw_in.ap().rearrange("(a p) n -> a p n", a=2)       # [2, 128, FWC]
    outv = mask_out.ap().rearrange("(h y) x -> h y x", y=UP)  # [64, 8, 1024]

    with tile.TileContext(nc) as tc:
        with (
            tc.tile_pool(name="persist", bufs=1) as pp,
            tc.tile_pool(name="work", bufs=3) as wp,
            tc.tile_pool(name="psA", bufs=3, space="PSUM") as psA,
            tc.tile_pool(name="psB", bufs=3, space="PSUM") as psB,
        ):
            fw0 = pp.tile([128, FWC], F16, tag="fw0")
            fw1 = pp.tile([128, FWC], F16, tag="fw1")
            biasv = pp.tile([1, 128], F32, tag="biasv")
            ones = pp.tile([1, CH], F32, tag="ones")
            ident = pp.tile([128, 128], F32, tag="ident")
            iota_i = pp.tile([128, K], I32, tag="iota_i")
            iotaf = pp.tile([128, K], F32, tag="iotaf")
            idxv = pp.tile([128, H], F32, tag="idxv")       # [w, (sg,c,j)]
            tph = pp.tile([128, H], F32, tag="tph")         # [w, h] linear
            tmp = pp.tile([128, H], F32, tag="tmp")         # block-transposed
            idxT = pp.tile([H, W], I8, tag="idxT")          # [h, w]
            rep = pp.tile([H, WL], I8, tag="rep")

            # --- setup ---
            nc.gpsimd.iota(iota_i, pattern=[[1, K]], base=0, channel_multiplier=0)
            nc.vector.tensor_copy(iotaf, iota_i)
            nc.gpsimd.memset(ones, 1.0)
            nc.sync.dma_start(biasv, bias_in[:, :])
            nc.scalar.dma_start(ident, ident_in[:, :])

            # --- feature loads: first supergroup split for a fast start ---
            ld_slices = [
                ds(0, 2 * KP + CH),            # w blocks + chunk 0
                ds(2 * KP + CH, CH),           # chunk 1
                ds(2 * KP + 2 * CH, 2 * CH),   # chunks 2-3 (sg0 tail)
            ]
            for sg in range(1, NSG):
                ld_slices.append(ds(2 * KP + sg * SG, SG))
            for i, sl in enumerate(ld_slices):
                eng = nc.sync if i % 2 == 0 else nc.scalar
                eng.dma_start(fw0[:, sl], fwv[0, :, sl])
                eng.dma_start(fw1[:, sl], fwv[1, :, sl])

            iota_b = iotaf.rearrange("p (g k) -> p g k", g=1, k=K).to_broadcast(
                [128, 16, K]
            )

            # --- per-supergroup: bias prefill + 16 col-tiled matmuls ->
            #     scores -> transposes -> argmax ---
            for sg in range(NSG):
                ps = psA.tile([128, CH], F32, tag="ps")
                nc.tensor.matmul(ps, biasv, ones, start=True, stop=False)
                for hf in range(2):
                    fwh = fw0 if hf == 0 else fw1
                    for part in range(2):
                        wsl = ds(part * KP, KP)
                        last = hf == 1 and part == 1
                        for j in range(4):
                            colsl = ds(2 * KP + sg * SG + j * CH, CH)
                            nc.tensor.matmul(
                                ps[ds(32 * j, 32), :],
                                fwh[:, wsl], fwh[:, colsl],
                                start=False, stop=last,
                                tile_position=(0, 32 * j),
                            )
                # plain ScalarE Copy is bit-exact
                S4 = wp.tile([128, CH], F32, tag="S4")
                nc.scalar.copy(S4, ps)
                psT = psB.tile([128, 4, 128], F32, tag="psT")
                for cc in range(4):
                    nc.tensor.transpose(
                        psT[:, cc], S4[:, ds(cc * 128, 128)], ident
                    )
                # argmax over k on strided 3D views (skip the 13 pad
                # cols; group g = 4*c + j has uniform stride 32)
                psTv = psT.rearrange("p a b -> p (a b)").rearrange(
                    "p (g k) -> p g k", g=16
                )[:, :, 0:K]
                maxv = wp.tile([128, 16], F32, tag="maxv")
                nc.vector.tensor_reduce(
                    maxv, psTv,
                    axis=mybir.AxisListType.X, op=mybir.AluOpType.max,
                )
                eq = wp.tile([128, 16, K], F32, tag="eq")
                maxv_b = maxv.rearrange("p (g o) -> p g o", o=1).to_broadcast(
                    [128, 16, K]
                )
                nc.vector.tensor_tensor(eq, psTv, maxv_b, op=mybir.AluOpType.is_ge)
                cand = wp.tile([128, 16, K], F32, tag="cand")
                nc.vector.scalar_tensor_tensor(
                    cand, eq, -BIG, iota_b,
                    op0=mybir.AluOpType.mult, op1=mybir.AluOpType.add,
                )
                nc.vector.tensor_reduce(
                    idxv[:, ds(sg * 16, 16)],
                    cand, axis=mybir.AxisListType.X, op=mybir.AluOpType.min,
                )

                # --- tail, overlapped: after each half of the supergroups,
                # emit that h-half of the output ---
                if sg % 2 != 1:
                    continue
                hh = sg // 2                   # 0 or 1
                hsl = ds(hh * 32, 32)          # 32 h columns
                psl = ds(hh * 32, 32)          # matching partition rows
                # idxv col order within each sg block is (c,j); true
                # h = sg*16 + j*4 + c.  Permuted read + +1024, 3D ops.
                for s in range(2):
                    sgi = 2 * hh + s
                    srcv = idxv[:, ds(sgi * 16, 16)].rearrange(
                        "p (c j) -> p c j", c=4
                    ).transpose([0, 2, 1])
                    nc.vector.tensor_scalar_add(
                        tph[:, ds(sgi * 16, 16)].rearrange(
                            "p (j c) -> p j c", j=4
                        ),
                        srcv, BIG,
                    )
                nc.vector.transpose(tmp[:, hsl], tph[:, hsl])
                for i in range(W // 32):
                    nc.vector.tensor_copy(
                        idxT[psl, ds(32 * i, 32)],
                        tmp[ds(32 * i, 32), hsl],
                    )
                # 8x x-replication on ScalarE (bit-exact copy engine-wise);
                # the 8x y-replication happens via 8 store DMAs re-reading
                # the same SBUF rows.
                idxT_b = idxT[psl].rearrange(
                    "p (w o) -> p w o", o=1
                ).to_broadcast([32, W, UP])
                nc.scalar.copy(
                    rep[psl].rearrange("p (w x) -> p w x", w=W), idxT_b
                )
                for y in range(UP):
                    deng = nc.sync if y % 2 == 0 else nc.scalar
                    deng.dma_start(outv[psl, y], rep[psl])

    nc.compile()
    return nc


_IDENT = None


def _prep_domain(feature, centroid):
    """Per-core inputs for one domain: 4 batches against one centroid set."""
    global _IDENT
    if _IDENT is None:
        _IDENT = np.ascontiguousarray(np.eye(128, dtype=np.float32))
    c = np.ascontiguousarray(centroid, dtype=np.float32)
    w = c.T.astype(np.float32)                                  # [C, K]
    w_hi = w.astype(np.float16)
    w_lo = (w.astype(np.float64) - w_hi.astype(np.float64)).astype(np.float16)
    # bias from the RECONSTRUCTED (quantized) centroids, in fp64
    chat = w_hi.astype(np.float64) + w_lo.astype(np.float64)    # [C, K]
    c2 = np.sum(chat * chat, axis=0)                            # [K]
    biasv = np.full((1, 128), NEG, dtype=np.float32)
    for j in range(4):
        biasv[0, 32 * j:32 * j + K] = (-0.5 * c2).astype(np.float32)
    wpad = np.zeros((C, 2 * KP), dtype=np.float16)
    wpad[:, 0:K] = w_hi
    wpad[:, KP:KP + K] = w_lo
    maps = []
    for b in range(B):
        f = np.asarray(feature[b], dtype=np.float32).reshape(C, NPIX)
        fw = np.ascontiguousarray(
            np.concatenate([wpad, f.astype(np.float16)], axis=1)
        )
        maps.append({"fw": fw, "bias": biasv, "ident": _IDENT})
    return maps


def kernel(
    feature_s2t, feature_target, label_s2t, label_target,
    centroid_s2t, centroid_target,
):
    global _NC_CACHE
    if _NC_CACHE is None:
        _NC_CACHE = _build_nc()
    nc = _NC_CACHE

    # cross assignment: s2t features vs target centroids, and vice versa
    in_maps = _prep_domain(feature_s2t, centroid_target) + _prep_domain(
        feature_target, centroid_s2t
    )
    res = run_bass_kernel_spmd(nc, in_maps, core_ids=list(range(8))).results
    mask_s2t = np.stack([res[i]["mask"] for i in range(B)]).astype(np.int32)
    mask_target = np.stack([res[B + i]["mask"] for i in range(B)]).astype(
        np.int32
    )
    return (mask_s2t, mask_target)


# revision 7
# speedup vs baseline: 1.4385x; 1.0854x over previous
"""VQ codebook assignment + nearest upsample on 8 NeuronCores.

Problem (per domain): given features f [B=4, C=256, H=64, W=128] and
centroids c [K=19, C=256], compute argmin_k ||f[b,:,h,w] - c_k||^2 and
nearest-upsample the [64,128] index map to [512,1024] (8x in each axis).
Two independent domains (cross-assigned centroids) x 4 batches = 8 cores,
one batch-image per core, no cross-core communication.

Precision scheme (measured on the actual inputs): features are quantized
to fp16 on the host (halves HBM traffic and PE passes); centroids are
carried as an fp16 hi+lo pair (w = hi + lo reconstructs fp32-fidelity
weights), and the bias -||c||^2/2 is computed on the host in fp64 from
the reconstructed centroids and added exactly via an fp32 PE matmul.
Scores stay fp32 throughout.  Measured rel_err 1.3e-2 (< 2e-2 gate);
bf16 or fp16-centroid variants measure over the gate and are not used.

Per-core pipeline, per supergroup of 4 x 512-pixel chunks:
  1. Bias prefill: matmul(ones[1,512], biasvec[1,128]) writes the
     per-k bias into all 512 psum columns (start=True opens the
     accumulation group; fp32 1.0*b is exact under LOW_HIGH).
  2. 16 fp16 matmuls accumulate cross[k,px]: the w hi/lo blocks (padded
     to 32 cols) are stationary; each of the 4 chunks streams through
     its own 32-col array strip (tile_position col tiling inferred from
     the psum slice base partition) so 4 matmuls run concurrently.
  3. One ScalarE copy moves the [128,512] scores PSUM->SBUF (plain Copy
     is bit-exact; the Identity-LUT bias path is not).
  4. 4 PE transposes flip [128,128] score slices into pixel-partition
     layout [128px, (4 strips x 32)] in PSUM.
  5. DVE argmax on strided views that skip the 13 pad columns per
     strip: reduce_max, is_ge, (eq*-1024+iota), reduce_min -- exact
     first-match tie semantics in fp32.
Tail per 32-row half: permuted +1024 copy fixes the (c,j) supergroup
ordering back to linear h, DVE 32x32 transpose + block copies build the
[h,w] int8 map, ScalarE broadcast-copies the 8x x-replication, and 8
HWDGE stores per half re-read the same SBUF rows for the 8x
y-replication.  The mask travels as int8 (indices 0..18, lossless) and
the host widens to int32.

Measured on trn2 (8 cores, NTFF): see test.py output.
"""

import numpy as np

import concourse.bass as bass
import concourse.mybir as mybir
import concourse.tile as tile
from concourse import bacc
from concourse.bass import ds
from concourse.bass_utils import run_bass_kernel_spmd

F32 = mybir.dt.float32
F16 = mybir.dt.float16
I32 = mybir.dt.int32
I8 = mybir.dt.int8

B = 4
C = 256
H, W = 64, 128
K = 19
KP = 32               # w block padded to 32 columns (one array strip)
HL, WL = 512, 1024
NPIX = H * W          # 8192
CH = 512              # matmul moving chunk (pixels)
SG = 4 * CH           # supergroup: 4 chunks processed concurrently
NSG = NPIX // SG      # 4 supergroups
UP = HL // H          # 8x upsample
BIG = 1024.0
NEG = -1.0e30         # pad-row bias; never wins the max
FWC = 2 * KP + NPIX   # fw columns: [w_hi32 | w_lo32 | pixels]

_NC_CACHE = None


def _build_nc():
    nc = bacc.Bacc("TRN2", target_bir_lowering=False, debug=False)

    fw_in = nc.dram_tensor("fw", [C, FWC], F16, kind="ExternalInput")
    bias_in = nc.dram_tensor("bias", [128, 2 * K], F32, kind="ExternalInput")
    ident_in = nc.dram_tensor("ident", [128, 128], F32, kind="ExternalInput")
    mask_out = nc.dram_tensor("mask", [HL, WL], I8, kind="ExternalOutput")

    fwv = fw_in.ap().rearrange("(a p) n -> a p n", a=2)       # [2, 128, FWC]
    outv = mask_out.ap().rearrange("(h y) x -> h y x", y=UP)  # [64, 8, 1024]

    with tile.TileContext(nc) as tc:
        with (
            tc.tile_pool(name="persist", bufs=1) as pp,
            tc.tile_pool(name="work", bufs=3) as wp,
            tc.tile_pool(name="psA", bufs=3, space="PSUM") as psA,
            tc.tile_pool(name="psB", bufs=3, space="PSUM") as psB,
        ):
            fw0 = pp.tile([128, FWC], F16, tag="fw0")
            fw1 = pp.tile([128, FWC], F16, tag="fw1")
            bias128 = pp.tile([128, K], F32, tag="bias128")
            ident = pp.tile([128, 128], F32, tag="ident")
            iotaf = pp.tile([128, K], F32, tag="iotaf")
            idxv = pp.tile([128, H], F32, tag="idxv")       # [w, (sg,c,j)]
            tph = pp.tile([128, H], F32, tag="tph")         # [w, h] linear
            tmp = pp.tile([128, H], F32, tag="tmp")         # block-transposed
            idxT = pp.tile([H, W], I8, tag="idxT")          # [h, w]
            rep = pp.tile([H, WL], I8, tag="rep")

            # --- feature loads first (critical path); small setup
            # tensors ride along after the first chunk ---
            ld_slices = [
                ds(0, 2 * KP + CH),            # w blocks + chunk 0
                ds(2 * KP + CH, CH),           # chunk 1
                ds(2 * KP + 2 * CH, 2 * CH),   # chunks 2-3 (sg0 tail)
            ]
            for sg in range(1, NSG):
                ld_slices.append(ds(2 * KP + sg * SG, SG))
            for i, sl in enumerate(ld_slices):
                eng = nc.sync if i % 2 == 0 else nc.scalar
                eng.dma_start(fw0[:, sl], fwv[0, :, sl])
                eng.dma_start(fw1[:, sl], fwv[1, :, sl])
                if i == 0:
                    nc.sync.dma_start(ident, ident_in[:, :])
                    nc.scalar.dma_start(
                        bias128, bias_in.ap()[:, 0:K]
                    )
                    nc.scalar.dma_start(
                        iotaf, bias_in.ap()[:, K:2 * K]
                    )

            iota_b = iotaf.rearrange("p (g k) -> p g k", g=1, k=K).to_broadcast(
                [128, 16, K]
            )

            # --- per-supergroup: bias prefill + 16 col-tiled matmuls ->
            #     scores -> transposes -> argmax ---
            for sg in range(NSG):
                ps = psA.tile([128, CH], F32, tag="ps")
                for hf in range(2):
                    fwh = fw0 if hf == 0 else fw1
                    for part in range(2):
                        wsl = ds(part * KP, KP)
                        first = hf == 0 and part == 0
                        last = hf == 1 and part == 1
                        for j in range(4):
                            colsl = ds(2 * KP + sg * SG + j * CH, CH)
                            nc.tensor.matmul(
                                ps[ds(32 * j, 32), :],
                                fwh[:, wsl], fwh[:, colsl],
                                start=first, stop=last,
                                tile_position=(0, 32 * j),
                            )
                # plain ScalarE Copy is bit-exact
                S4 = wp.tile([128, CH], F32, tag="S4")
                nc.scalar.copy(S4, ps)
                psT = psB.tile([128, 4, 128], F32, tag="psT")
                for cc in range(4):
                    nc.tensor.transpose(
                        psT[:, cc], S4[:, ds(cc * 128, 128)], ident
                    )
                # argmax over k on strided 3D views (skip the 13 pad
                # cols; group g = 4*c + j has uniform stride 32).  The
                # exact fp32 bias add rides the PSUM->SBUF move.
                psTv = psT.rearrange("p a b -> p (a b)").rearrange(
                    "p (g k) -> p g k", g=16
                )[:, :, 0:K]
                bias_b = bias128.rearrange(
                    "p (g k) -> p g k", g=1, k=K
                ).to_broadcast([128, 16, K])
                S4b = wp.tile([128, 16, K], F32, tag="S4b")
                nc.vector.tensor_tensor(S4b, psTv, bias_b, op=mybir.AluOpType.add)
                maxv = wp.tile([128, 16], F32, tag="maxv")
                nc.vector.tensor_reduce(
                    maxv, S4b,
                    axis=mybir.AxisListType.X, op=mybir.AluOpType.max,
                )
                eq = wp.tile([128, 16, K], F32, tag="eq")
                maxv_b = maxv.rearrange("p (g o) -> p g o", o=1).to_broadcast(
                    [128, 16, K]
                )
                nc.vector.tensor_tensor(eq, S4b, maxv_b, op=mybir.AluOpType.is_ge)
                cand = wp.tile([128, 16, K], F32, tag="cand")
                nc.vector.scalar_tensor_tensor(
                    cand, eq, -BIG, iota_b,
                    op0=mybir.AluOpType.mult, op1=mybir.AluOpType.add,
                )
                nc.vector.tensor_reduce(
                    idxv[:, ds(sg * 16, 16)],
                    cand, axis=mybir.AxisListType.X, op=mybir.AluOpType.min,
                )

                # --- tail, overlapped: after each half of the supergroups,
                # emit that h-half of the output ---
                if sg % 2 != 1:
                    continue
                hh = sg // 2                   # 0 or 1
                hsl = ds(hh * 32, 32)          # 32 h columns
                psl = ds(hh * 32, 32)          # matching partition rows
                # idxv col order within each sg block is (c,j); true
                # h = sg*16 + j*4 + c.  Permuted read + +1024, 3D ops.
                for s in range(2):
                    sgi = 2 * hh + s
                    srcv = idxv[:, ds(sgi * 16, 16)].rearrange(
                        "p (c j) -> p c j", c=4
                    ).transpose([0, 2, 1])
                    nc.vector.tensor_scalar_add(
                        tph[:, ds(sgi * 16, 16)].rearrange(
                            "p (j c) -> p j c", j=4
                        ),
                        srcv, BIG,
                    )
                nc.vector.transpose(tmp[:, hsl], tph[:, hsl])
                for i in range(W // 32):
                    nc.vector.tensor_copy(
                        idxT[psl, ds(32 * i, 32)],
                        tmp[ds(32 * i, 32), hsl],
                    )
                # 8x x-replication on ScalarE (bit-exact copy engine-wise);
                # the 8x y-replication happens via 8 store DMAs re-reading
                # the same SBUF rows.
                idxT_b = idxT[psl].rearrange(
                    "p (w o) -> p w o", o=1
                ).to_broadcast([32, W, UP])
                nc.scalar.copy(
                    rep[psl].rearrange("p (w x) -> p w x", w=W), idxT_b
                )
                for y in range(UP):
                    deng = nc.sync if y % 2 == 0 else nc.scalar
                    deng.dma_start(outv[psl, y], rep[psl])

    nc.compile()
    return nc


_IDENT = None


def _prep_domain(feature, centroid):
    """Per-core inputs for one domain: 4 batches against one centroid set."""
    global _IDENT
    if _IDENT is None:
        _IDENT = np.ascontiguousarray(np.eye(128, dtype=np.float32))
    c = np.ascontiguousarray(centroid, dtype=np.float32)
    w = c.T.astype(np.float32)                                  # [C, K]
    w_hi = w.astype(np.float16)
    w_lo = (w.astype(np.float64) - w_hi.astype(np.float64)).astype(np.float16)
    # bias from the RECONSTRUCTED (quantized) centroids, in fp64
    chat = w_hi.astype(np.float64) + w_lo.astype(np.float64)    # [C, K]
    c2 = np.sum(chat * chat, axis=0)                            # [K]
    bi = np.zeros((128, 2 * K), dtype=np.float32)
    bi[:, 0:K] = (-0.5 * c2).astype(np.float32)[None, :]
    bi[:, K:2 * K] = np.arange(K, dtype=np.float32)[None, :]
    wpad = np.zeros((C, 2 * KP), dtype=np.float16)
    wpad[:, 0:K] = w_hi
    wpad[:, KP:KP + K] = w_lo
    maps = []
    for b in range(B):
        f = np.asarray(feature[b], dtype=np.float32).reshape(C, NPIX)
        fw = np.ascontiguousarray(
            np.concatenate([wpad, f.astype(np.float16)], axis=1)
        )
        maps.append({"fw": fw, "bias": bi, "ident": _IDENT})
    return maps


def kernel(
    feature_s2t, feature_target, label_s2t, label_target,
    centroid_s2t, centroid_target,
):
    global _NC_CACHE
    if _NC_CACHE is None:
        _NC_CACHE = _build_nc()
    nc = _NC_CACHE

    # cross assignment: s2t features vs target centroids, and vice versa
    in_maps = _prep_domain(feature_s2t, centroid_target) + _prep_domain(
        feature_target, centroid_s2t
    )
    res = run_bass_kernel_spmd(nc, in_maps, core_ids=list(range(8))).results
    mask_s2t = np.stack([res[i]["mask"] for i in range(B)]).astype(np.int32)
    mask_target = np.stack([res[B + i]["mask"] for i in range(B)]).astype(
        np.int32
    )
    return (mask_s2t, mask_target)


# revision 11
# speedup vs baseline: 1.6350x; 1.1366x over previous
"""VQ codebook assignment + nearest upsample on 8 NeuronCores.

Problem (per domain): given features f [B=4, C=256, H=64, W=128] and
centroids c [K=19, C=256], compute argmin_k ||f[b,:,h,w] - c_k||^2 and
nearest-upsample the [64,128] index map to [512,1024] (8x in each axis).
Two independent domains (cross-assigned centroids) x 4 batches = 8 cores,
one batch-image per core, no cross-core communication.

Precision scheme (measured on the actual inputs): features are quantized
to fp16 on the host (halves HBM traffic and PE passes); centroids are
carried as an fp16 hi+lo pair (w = hi + lo reconstructs fp32-fidelity
weights), and the bias -||c||^2/2 is computed on the host in fp64 from
the reconstructed centroids and added exactly via an fp32 PE matmul.
Scores stay fp32 throughout.  Measured rel_err 1.3e-2 (< 2e-2 gate);
bf16 or fp16-centroid variants measure over the gate and are not used.

Per-core pipeline, per supergroup of 4 x 512-pixel chunks:
  1. Bias prefill: matmul(ones[1,512], biasvec[1,128]) writes the
     per-k bias into all 512 psum columns (start=True opens the
     accumulation group; fp32 1.0*b is exact under LOW_HIGH).
  2. 16 fp16 matmuls accumulate cross[k,px]: the w hi/lo blocks (padded
     to 32 cols) are stationary; each of the 4 chunks streams through
     its own 32-col array strip (tile_position col tiling inferred from
     the psum slice base partition) so 4 matmuls run concurrently.
  3. One ScalarE copy moves the [128,512] scores PSUM->SBUF (plain Copy
     is bit-exact; the Identity-LUT bias path is not).
  4. 4 PE transposes flip [128,128] score slices into pixel-partition
     layout [128px, (4 strips x 32)] in PSUM.
  5. DVE argmax on strided views that skip the 13 pad columns per
     strip: reduce_max, is_ge, (eq*-1024+iota), reduce_min -- exact
     first-match tie semantics in fp32.
Tail per 32-row half: permuted +1024 copy fixes the (c,j) supergroup
ordering back to linear h, DVE 32x32 transpose + block copies build the
[h,w] int8 map, ScalarE broadcast-copies the 8x x-replication, and 8
HWDGE stores per half re-read the same SBUF rows for the 8x
y-replication.  The mask travels as int8 (indices 0..18, lossless) and
the host widens to int32.

Measured on trn2 (8 cores, NTFF): see test.py output.
"""

import numpy as np

import concourse.bass as bass
import concourse.mybir as mybir
import concourse.tile as tile
from concourse import bacc
from concourse.bass import ds
from concourse.bass_utils import run_bass_kernel_spmd

F32 = mybir.dt.float32
F16 = mybir.dt.float16
I32 = mybir.dt.int32
I8 = mybir.dt.int8

B = 4
C = 256
H, W = 64, 128
K = 19
KP = 32               # w block padded to 32 columns (one array strip)
HL, WL = 512, 1024
NPIX = H * W          # 8192
CH = 512              # matmul moving chunk (pixels)
SG = 4 * CH           # supergroup: 4 chunks processed concurrently
NSG = NPIX // SG      # 4 supergroups
UP = HL // H          # 8x upsample
BIG = 1024.0
NEG = -1.0e30         # pad-row bias; never wins the max
FWC = 2 * KP + NPIX   # fw columns: [w_hi32 | w_lo32 | pixels]

_NC_CACHE = None


def _build_nc():
    nc = bacc.Bacc("TRN2", target_bir_lowering=False, debug=False)

    fw_in = nc.dram_tensor("fw", [C, FWC], F16, kind="ExternalInput")
    bias_in = nc.dram_tensor("bias", [128, 2 * K], F32, kind="ExternalInput")
    ident_in = nc.dram_tensor("ident", [128, 128], F32, kind="ExternalInput")
    mask_out = nc.dram_tensor("mask", [HL, WL], I8, kind="ExternalOutput")

    fwv = fw_in.ap().rearrange("(a p) n -> a p n", a=2)       # [2, 128, FWC]
    outv = mask_out.ap().rearrange("(h y) x -> h y x", y=UP)  # [64, 8, 1024]

    with tile.TileContext(nc) as tc:
        with (
            tc.tile_pool(name="persist", bufs=1) as pp,
            tc.tile_pool(name="work", bufs=3) as wp,
            tc.tile_pool(name="psA", bufs=3, space="PSUM") as psA,
            tc.tile_pool(name="psB", bufs=3, space="PSUM") as psB,
            tc.tile_pool(name="psS", bufs=1, space="PSUM") as psS,
        ):
            fw0 = pp.tile([128, FWC], F16, tag="fw0")
            fw1 = pp.tile([128, FWC], F16, tag="fw1")
            bias128 = pp.tile([128, K], F32, tag="bias128")
            ident = pp.tile([128, 128], F32, tag="ident")
            iotaf = pp.tile([128, K], F32, tag="iotaf")
            idxv = pp.tile([128, H], F32, tag="idxv")       # [w, (sg,c,j)]
            tph = pp.tile([128, H], F32, tag="tph")         # [w, h] linear
            tmp = pp.tile([128, H], F32, tag="tmp")         # block-transposed
            idxT = pp.tile([H, W], I8, tag="idxT")          # [h, w]
            rep = pp.tile([H, WL], I8, tag="rep")

            # --- PE warm-up: dummy matmuls on (uninitialized) scratch
            # keep the HAM activity monitor busy through the DMA-in phase
            # so the real matmul stream runs at 2.4 GHz, not 1.2 ---
            scr = pp.tile([128, CH], F16, tag="scr")
            nc.scalar.memzero(scr)
            pssc = psS.tile([128, CH], F32, tag="pssc")
            for _ in range(18):
                nc.tensor.matmul(
                    pssc[ds(0, 32), :], scr[:, 0:32], scr,
                    start=True, stop=True, skip_group_check=True,
                )

            # --- feature loads, all on the sync HWDGE queue: the scalar
            # queue must stay clear for the per-supergroup ScalarE copies
            # (queue = sequencer FIFO; a copy behind 10 load triggers
            # serializes the whole pipeline behind the load phase).
            # Setup tensors load via the scalar queue up front. ---
            nc.scalar.dma_start(ident, ident_in[:, :])
            nc.scalar.dma_start(bias128, bias_in.ap()[:, 0:K])
            nc.scalar.dma_start(iotaf, bias_in.ap()[:, K:2 * K])
            ld_slices = [
                ds(0, 2 * KP + CH),            # w blocks + chunk 0
                ds(2 * KP + CH, CH),           # chunk 1
                ds(2 * KP + 2 * CH, 2 * CH),   # chunks 2-3 (sg0 tail)
            ]
            for sg in range(1, NSG):
                ld_slices.append(ds(2 * KP + sg * SG, SG))
            for i, sl in enumerate(ld_slices):
                nc.sync.dma_start(fw0[:, sl], fwv[0, :, sl])
                nc.sync.dma_start(fw1[:, sl], fwv[1, :, sl])

            iota_b = iotaf.rearrange("p (g k) -> p g k", g=1, k=K).to_broadcast(
                [128, 16, K]
            )

            # --- per-supergroup: bias prefill + 16 col-tiled matmuls ->
            #     scores -> transposes -> argmax ---
            for sg in range(NSG):
                ps = psA.tile([128, CH], F32, tag="ps")
                for hf in range(2):
                    fwh = fw0 if hf == 0 else fw1
                    for part in range(2):
                        wsl = ds(part * KP, KP)
                        first = hf == 0 and part == 0
                        last = hf == 1 and part == 1
                        for j in range(4):
                            colsl = ds(2 * KP + sg * SG + j * CH, CH)
                            nc.tensor.matmul(
                                ps[ds(32 * j, 32), :],
                                fwh[:, wsl], fwh[:, colsl],
                                start=first, stop=last,
                                tile_position=(0, 32 * j),
                            )
                # plain ScalarE Copy is bit-exact
                S4 = wp.tile([128, CH], F32, tag="S4")
                nc.scalar.copy(S4, ps)
                psT = psB.tile([128, 4, 128], F32, tag="psT")
                for cc in range(4):
                    nc.tensor.transpose(
                        psT[:, cc], S4[:, ds(cc * 128, 128)], ident
                    )
                # argmax over k on strided 3D views (skip the 13 pad
                # cols; group g = 4*c + j has uniform stride 32).  The
                # exact fp32 bias add rides the PSUM->SBUF move.
                psTv = psT.rearrange("p a b -> p (a b)").rearrange(
                    "p (g k) -> p g k", g=16
                )[:, :, 0:K]
                bias_b = bias128.rearrange(
                    "p (g k) -> p g k", g=1, k=K
                ).to_broadcast([128, 16, K])
                S4b = wp.tile([128, 16, K], F32, tag="S4b")
                nc.vector.tensor_tensor(S4b, psTv, bias_b, op=mybir.AluOpType.add)
                maxv = wp.tile([128, 16], F32, tag="maxv")
                nc.vector.tensor_reduce(
                    maxv, S4b,
                    axis=mybir.AxisListType.X, op=mybir.AluOpType.max,
                )
                eq = wp.tile([128, 16, K], F32, tag="eq")
                maxv_b = maxv.rearrange("p (g o) -> p g o", o=1).to_broadcast(
                    [128, 16, K]
                )
                nc.vector.tensor_tensor(eq, S4b, maxv_b, op=mybir.AluOpType.is_ge)
                cand = wp.tile([128, 16, K], F32, tag="cand")
                nc.vector.scalar_tensor_tensor(
                    cand, eq, -BIG, iota_b,
                    op0=mybir.AluOpType.mult, op1=mybir.AluOpType.add,
                )
                nc.vector.tensor_reduce(
                    idxv[:, ds(sg * 16, 16)],
                    cand, axis=mybir.AxisListType.X, op=mybir.AluOpType.min,
                )

                # --- tail, overlapped: after each half of the supergroups,
                # emit that h-half of the output ---
                if sg % 2 != 1:
                    continue
                hh = sg // 2                   # 0 or 1
                hsl = ds(hh * 32, 32)          # 32 h columns
                psl = ds(hh * 32, 32)          # matching partition rows
                # idxv col order within each sg block is (c,j); true
                # h = sg*16 + j*4 + c.  Permuted read + +1024, 3D ops.
                for s in range(2):
                    sgi = 2 * hh + s
                    srcv = idxv[:, ds(sgi * 16, 16)].rearrange(
                        "p (c j) -> p c j", c=4
                    ).transpose([0, 2, 1])
                    nc.vector.tensor_scalar_add(
                        tph[:, ds(sgi * 16, 16)].rearrange(
                            "p (j c) -> p j c", j=4
                        ),
                        srcv, BIG,
                    )
                nc.vector.transpose(tmp[:, hsl], tph[:, hsl])
                for i in range(W // 32):
                    nc.vector.tensor_copy(
                        idxT[psl, ds(32 * i, 32)],
                        tmp[ds(32 * i, 32), hsl],
                    )
                # 8x x-replication on ScalarE (bit-exact copy engine-wise);
                # the 8x y-replication happens via 8 store DMAs re-reading
                # the same SBUF rows.
                idxT_b = idxT[psl].rearrange(
                    "p (w o) -> p w o", o=1
                ).to_broadcast([32, W, UP])
                nc.scalar.copy(
                    rep[psl].rearrange("p (w x) -> p w x", w=W), idxT_b
                )
                rep_b = rep[psl].rearrange(
                    "p (o x) -> p o x", o=1
                ).to_broadcast([32, UP, WL])
                nc.sync.dma_start(outv[psl], rep_b)

    nc.compile()
    return nc


_IDENT = None


def _prep_domain(feature, centroid):
    """Per-core inputs for one domain: 4 batches against one centroid set."""
    global _IDENT
    if _IDENT is None:
        _IDENT = np.ascontiguousarray(np.eye(128, dtype=np.float32))
    c = np.ascontiguousarray(centroid, dtype=np.float32)
    w = c.T.astype(np.float32)                                  # [C, K]
    w_hi = w.astype(np.float16)
    w_lo = (w.astype(np.float64) - w_hi.astype(np.float64)).astype(np.float16)
    # bias from the RECONSTRUCTED (quantized) centroids, in fp64
    chat = w_hi.astype(np.float64) + w_lo.astype(np.float64)    # [C, K]
    c2 = np.sum(chat * chat, axis=0)                            # [K]
    bi = np.zeros((128, 2 * K), dtype=np.float32)
    bi[:, 0:K] = (-0.5 * c2).astype(np.float32)[None, :]
    bi[:, K:2 * K] = np.arange(K, dtype=np.float32)[None, :]
    wpad = np.zeros((C, 2 * KP), dtype=np.float16)
    wpad[:, 0:K] = w_hi
    wpad[:, KP:KP + K] = w_lo
    maps = []
    for b in range(B):
        f = np.asarray(feature[b], dtype=np.float32).reshape(C, NPIX)
        fw = np.ascontiguousarray(
            np.concatenate([wpad, f.astype(np.float16)], axis=1)
        )
        maps.append({"fw": fw, "bias": bi, "ident": _IDENT})
    return maps


def kernel(
    feature_s2t, feature_target, label_s2t, label_target,
    centroid_s2t, centroid_target,
):
    global _NC_CACHE
    if _NC_CACHE is None:
        _NC_CACHE = _build_nc()
    nc = _NC_CACHE

    # cross assignment: s2t features vs target centroids, and vice versa
    in_maps = _prep_domain(feature_s2t, centroid_target) + _prep_domain(
        feature_target, centroid_s2t
    )
    res = run_bass_kernel_spmd(nc, in_maps, core_ids=list(range(8))).results
    mask_s2t = np.stack([res[i]["mask"] for i in range(B)]).astype(np.int32)
    mask_target = np.stack([res[B + i]["mask"] for i in range(B)]).astype(
        np.int32
    )
    return (mask_s2t, mask_target)
